# revision 36
# baseline (speedup 1.0000x reference)
"""Trainium2 Bass kernel for the 2-player masked LSTM scan.

Reference semantics (T=128 steps, B=256 batch, C=1024 in, H=1024 hidden):
  per step t, batch b: the active player's (c,h) (selected by main[t,b]) runs
  one LSTM cell z = x@Wi + h@Wh + b with fused i,f,g,o gates; the result is
  written back only to the active player's state, and both players' states are
  zeroed where done[t,b].

Key algorithmic idea: done/main are *inputs*, so the true dependency structure
is known on the host.  Each (b, segment, player) triple forms an independent
"chain" of positions; a position at depth d in its chain depends only on the
position at depth d-1.  With done ~ Bernoulli(0.5) per step, chains are short
(max depth ~17 for the target inputs), so the sequential scan of 128 steps
becomes ~17 dense "waves", each a full-batch matmul with no masking at all.

Host side: sort chains by length (desc), round-robin across the 8 cores, lay
positions out wave-major.  Because chain order is sorted by length, the chains
alive at depth d are exactly a prefix of those alive at depth d-1 - so wave d
reads a contiguous prefix of wave d-1's outputs: no gather needed on device.

Device: phase A computes zx = x@Wi (+ bias via DVE) for all positions, fusing
the full gate math for depth-0 positions (input state is zero); phase B runs
one matmul z = zx + h@Wh per wave plus the LSTM gate math.  bf16 lhsT (h^T)
x fp8e4 rhs (Wh) matmuls with fp32 PSUM accumulation (fp8 Wh halves SBUF +
DMA and is fully prefetched during phase A); the carried cell state c stays
fp32.  Wave-1's h^T is transposed incrementally DURING phase A straight from
SBUF nhb tiles, and each wave pre-builds its successor's h^T the same way,
so no wave ever waits on a bulk xbar transpose.

Waves with <=64 live rows (the sequential tail, where a full-width matmul
would be bound by streaming all of Wh through the PE) instead run a packed
col-tiled pipeline: the 4 gates' z chunks are computed CONCURRENTLY on the
PE's four 32-wide column strips (tile_position col tiling, ~4x the Wh
streaming rate) into one packed PSUM tile [4*32, 512]; zx is folded in by
the PSUM->SBUF move; PE transposes then put h-columns on partitions so the
tiny row count is the free dim - gate math runs on [128, 4, 32] strided
views and its nhT output IS the next wave's stationary operand.  Tail y is
written transposed (yt) and unpacked on the host.

Wave-1 remainder rows whose chains die at depth 1 ride in wave 2's
last-tile padding instead of paying their own m-tile (_merge12): their
h^T is DVE-copied out of hT1 into a 16-aligned column slot kept disjoint
from the prebuilt-hT transposes, zx/c arrive by two small DMA overwrites,
and the host remaps their y rows - wave 1 drops a full 13.8us tile.
"""

import sys

sys.path.insert(0, "/opt/trn_rl_repo")

import numpy as np
import ml_dtypes

import concourse.bass as bass
import concourse.tile as tile
from concourse import bacc, mybir
from concourse.bass_utils import run_bass_kernel_spmd

BF16 = ml_dtypes.bfloat16
FP8 = ml_dtypes.float8_e4m3
AF = mybir.ActivationFunctionType
DT = mybir.dt

NCORES = 8
H = 1024
CIN = 1024
G = 4 * H  # 4096 fused gate width
KT = CIN // 128  # 8 k-tiles for both Wi and Wh contractions

# Phase-A gates computed with fp8 DoubleRow matmuls (2x PE rate) instead of
# bf16.  Error budget (measured by exact CPU simulation of the quantization
# pipeline vs the fp32 reference, tolerance 2e-2):
#   ""    -> 6.6e-3   (phase B DR only)
#   "f"   -> 8.0e-3   (f is nearly free: 70% of positions are depth-0 where
#                      c=0 and the forget gate is multiplied by zero)
#   "if"  -> 1.64e-2
#   "g" is untouchable (tanh slope 1): 4.4e-2 alone.
PHASEA_F8 = "if"
GATE_IDX = {"i": 0, "f": 1, "g": 2, "o": 3}


# ---------------------------------------------------------------------------
# Host-side schedule construction
# ---------------------------------------------------------------------------

def _build_schedule(done, main, T, B):
    """Chain decomposition of the (t, b) grid.

    Returns per-position (core, depth, rank) and the uniform padded wave
    geometry shared by all cores (SPMD requires identical programs).
    """
    done2 = done.reshape(T, B).astype(bool)
    main2 = main.reshape(T, B).astype(bool)

    seg = np.zeros((T, B), np.int64)
    if T > 1:
        seg[1:] = np.cumsum(done2[:-1], axis=0)
    player = main2.astype(np.int64)
    key = (np.arange(B)[None, :] * (T + 1) + seg) * 2 + player  # [T, B]
    flat_key = key.reshape(-1)  # position p = t*B + b
    order = np.argsort(flat_key, kind="stable")  # chain-major, t-ascending
    sorted_keys = flat_key[order]
    uk, first_idx, inv = np.unique(sorted_keys, return_index=True, return_inverse=True)
    chain_len = np.diff(np.append(first_idx, len(sorted_keys)))
    npos = T * B

    depth = np.empty(npos, np.int64)
    depth[order] = np.arange(npos) - first_idx[inv]
    chain_id = np.empty(npos, np.int64)
    chain_id[order] = inv

    n_chains = len(uk)
    chain_b = uk // (2 * (T + 1))
    chain_seg = (uk // 2) % (T + 1)
    chain_player = uk % 2

    chain_order = np.argsort(-chain_len, kind="stable")
    rank_of_chain = np.empty(n_chains, np.int64)
    rank_of_chain[chain_order] = np.arange(n_chains)
    core_of_chain = (rank_of_chain % NCORES).astype(np.int64)
    core_rank = rank_of_chain // NCORES

    D = int(chain_len.max())
    lens_sorted = np.sort(chain_len)
    N_d = np.array([n_chains - np.searchsorted(lens_sorted, d, side="right")
                    for d in range(D)], np.int64)
    U = np.ceil(N_d / NCORES).astype(np.int64)      # uniform per-core wave rows
    M = np.ceil(U / 128).astype(np.int64)           # padded wave m-tiles
    V = np.concatenate([[0], np.cumsum(U)])          # packed row offsets
    P = np.concatenate([[0], np.cumsum(M * 128)])    # padded row offsets

    return dict(
        depth=depth, chain_id=chain_id, core_of_chain=core_of_chain,
        core_rank=core_rank, chain_b=chain_b, chain_seg=chain_seg,
        chain_player=chain_player, D=D, U=U, M=M, V=V, P=P,
    )


def _prep_inputs(x, c1, h1, c2, h2, Wi, Wh, b, done, main):
    """Build per-core device input arrays + output scatter indices."""
    B = c1.shape[0]
    T = x.shape[0] // B
    sch = _build_schedule(np.asarray(done), np.asarray(main), T, B)
    D, U, M, V, P = sch["D"], sch["U"], sch["M"], sch["V"], sch["P"]

    zero_init = not (np.any(c1) or np.any(h1) or np.any(c2) or np.any(h2))

    packed_total = int(V[D])
    need = packed_total
    for d in range(1 if zero_init else 0, D):
        need = max(need, int(V[d]) + int(M[d]) * 128)
    Mzx = (need + 127) // 128
    zx_row0 = int(V[1]) // 128 * 128 if (zero_init and D > 1) else 0
    zx_start_tile = zx_row0 // 128

    depth = sch["depth"]; chain_id = sch["chain_id"]
    core_pos = sch["core_of_chain"][chain_id]
    packed_row = V[depth] + sch["core_rank"][chain_id]
    padded_row = P[depth] + sch["core_rank"][chain_id]

    f8_gates = [g for g in "ifgo" if g in PHASEA_F8]
    bf_gates = [g for g in "ifgo" if g not in PHASEA_F8]

    x = np.ascontiguousarray(np.asarray(x, np.float32))
    xt_blocks = []
    xt8_blocks = []
    for c in range(NCORES):
        sel = core_pos == c
        Xp = np.zeros((Mzx * 128, CIN), np.float32)
        Xp[packed_row[sel]] = x[sel]
        # lhsT block layout: [mt, p, k*128 + m] = Xp[mt*128+m, k*128+p]
        xt = Xp.reshape(Mzx, 128, KT, 128).transpose(0, 3, 2, 1).reshape(Mzx, 128, CIN)
        xt_blocks.append(np.ascontiguousarray(xt.astype(BF16)))
        if f8_gates:
            xt8_blocks.append(np.ascontiguousarray(xt.astype(FP8)))

    # weight layout per k-slice: [k, p, n] = W[k*128+p, n].  Wi is split by
    # gate precision: bf16 columns for bf_gates, fp8 columns for f8_gates.
    Wi_f = np.asarray(Wi, np.float32).reshape(KT, 128, 4, H)
    Wi_l = np.ascontiguousarray(
        Wi_f[:, :, [GATE_IDX[g] for g in bf_gates], :]
        .reshape(KT, 128, len(bf_gates) * H).astype(BF16))
    Wi8_l = None
    if f8_gates:
        Wi8_l = np.ascontiguousarray(
            Wi_f[:, :, [GATE_IDX[g] for g in f8_gates], :]
            .reshape(KT, 128, len(f8_gates) * H).astype(FP8))
    Wh_l = np.ascontiguousarray(
        np.asarray(Wh, np.float32).reshape(KT, 128, G).astype(FP8))
    bbc = np.ascontiguousarray(
        np.broadcast_to(np.asarray(b, np.float32)[None, :], (128, G)).astype(BF16))
    ident = np.ascontiguousarray(np.eye(128, dtype=np.float32).astype(BF16))
    identf = np.ascontiguousarray(np.eye(128, dtype=np.float32))

    ht0_blocks = [None] * NCORES
    c0_blocks = [None] * NCORES
    if not zero_init:
        h1 = np.asarray(h1, np.float32); h2 = np.asarray(h2, np.float32)
        c1 = np.asarray(c1, np.float32); c2 = np.asarray(c2, np.float32)
        hin = np.where(sch["chain_player"][:, None] > 0, h1[sch["chain_b"]],
                       h2[sch["chain_b"]])
        cin_ = np.where(sch["chain_player"][:, None] > 0, c1[sch["chain_b"]],
                        c2[sch["chain_b"]])
        live = sch["chain_seg"] == 0
        hin = np.where(live[:, None], hin, 0.0)
        cin_ = np.where(live[:, None], cin_, 0.0)
        M0 = int(M[0])
        for c in range(NCORES):
            selc = sch["core_of_chain"] == c
            rows = sch["core_rank"][selc]
            Hp = np.zeros((M0 * 128, H), np.float32)
            Cp = np.zeros((M0 * 128, H), np.float32)
            Hp[rows] = hin[selc]
            Cp[rows] = cin_[selc]
            # transposed layout for lhsT: [k, p, col] = Hp[col, k*128+p]
            ht0 = Hp.reshape(M0 * 128, KT, 128).transpose(1, 2, 0).astype(BF16)
            ht0_blocks[c] = np.ascontiguousarray(ht0)
            c0_blocks[c] = np.ascontiguousarray(Cp)

    return dict(
        sch=sch, zero_init=zero_init, Mzx=Mzx, zx_row0=zx_row0,
        zx_start_tile=zx_start_tile, xt_blocks=xt_blocks, Wi_l=Wi_l, Wh_l=Wh_l,
        xt8_blocks=xt8_blocks, Wi8_l=Wi8_l,
        bbc=bbc, ident=ident, identf=identf, ht0_blocks=ht0_blocks,
        c0_blocks=c0_blocks, core_pos=core_pos, padded_row=padded_row,
        T=T, B=B,
    )


# ---------------------------------------------------------------------------
# Device program
# ---------------------------------------------------------------------------

def _rem_plan(D, U, M, dp, zero_init):
    """Last-m-tile remainders (<=64 live rows, no state consumers) that can
    run through the packed col-tiled pipeline instead of a full-width tile."""
    d_start = 1 if zero_init else 0
    plan = []
    # Disabled: measured SLOWER on HW than the full-width tile it replaces
    # (728-745us vs 720us) - the col-tiled rounds' mode switches and serial
    # gate chain cost more than the padded-tile streaming they save.
    return plan
    for d in range(max(d_start, 1), min(D, dp)):
        Md = int(M[d])
        rem = int(U[d]) - (Md - 1) * 128
        nxt = int(U[d + 1]) if d + 1 < D else 0
        if Md >= 2 and 0 < rem <= 64 and nxt <= (Md - 1) * 128:
            plan.append((d, rem))
    return plan


def _merge12(D, U, M, dp, zero_init):
    """Rows of wave 1's last m-tile that can ride in wave 2's padding.

    Wave-1 remainder rows whose chains die at depth 1 (rank >= U[2]) need
    only y; their input h is the depth-0 output already held transposed in
    hT1.  If they fit in wave 2's last-tile padding, wave 1 drops a whole
    m-tile of Wh streaming.  Returns the remainder row count (0 = no merge).
    """
    if not zero_init or D <= 2 or dp <= 2:
        return 0
    M1, M2 = int(M[1]), int(M[2])
    if M1 < 2 or (M2 == 1 and M1 <= 3):  # wave2 must be a DRAM-path wave
        return 0
    rem1 = int(U[1]) - (M1 - 1) * 128
    ext0 = ((int(U[2]) + 15) // 16) * 16  # 16-aligned slot for the extras
    if 0 < rem1 <= M2 * 128 - ext0 and int(U[2]) <= (M1 - 1) * 128 \
            and ext0 >= (M2 - 1) * 128:
        return rem1
    return 0


def _compute_dp(D, U, M, zero_init):
    """First wave handled by the packed col-tiled tail pipeline."""
    d_start = 1 if zero_init else 0
    for d in range(max(d_start + 2, 2), D):
        if int(U[d]) <= 64 and int(M[d]) == 1:
            return d
    return D


def _build_program(D, U, M, V, P, Mzx, zx_row0, zx_start_tile, zero_init,
                   no_tail=False):
    nc = bacc.Bacc("TRN2", target_bir_lowering=False, debug=False)

    M0 = int(M[0])
    Ptot = int(P[D])
    d_start = 1 if zero_init else 0
    need_zx = D > d_start
    nzx_rows = Mzx * 128 - zx_row0
    dp = _compute_dp(D, U, M, zero_init) if not no_tail else D
    n_packed = D - dp
    merge_rem = 0 if no_tail else _merge12(D, U, M, dp, zero_init)

    f8_gates = [GATE_IDX[g] for g in "ifgo" if g in PHASEA_F8]
    bf_gates = [GATE_IDX[g] for g in "ifgo" if g not in PHASEA_F8]
    GBF = len(bf_gates) * H
    GF8 = len(f8_gates) * H

    xt_d = nc.dram_tensor("xt", [Mzx, 128, CIN], DT.bfloat16, kind="ExternalInput")
    wi_d = nc.dram_tensor("wi", [KT, 128, GBF], DT.bfloat16, kind="ExternalInput")
    if f8_gates:
        xt8_d = nc.dram_tensor("xt8", [Mzx, 128, CIN], DT.float8e4,
                               kind="ExternalInput")
        wi8_d = nc.dram_tensor("wi8", [KT, 128, GF8], DT.float8e4,
                               kind="ExternalInput")
    wh_d = nc.dram_tensor("wh", [KT, 128, G], DT.float8e4, kind="ExternalInput")
    bbc_d = nc.dram_tensor("bbc", [128, G], DT.bfloat16, kind="ExternalInput")
    id_d = nc.dram_tensor("ident", [128, 128], DT.bfloat16, kind="ExternalInput")
    idf_d = nc.dram_tensor("identf", [128, 128], DT.float32, kind="ExternalInput")
    y_d = nc.dram_tensor("y", [Ptot, H], DT.bfloat16, kind="ExternalOutput")
    if n_packed > 0:
        # packed-tail output, transposed: yt[w, kt, hcol%128, row]
        yt_d = nc.dram_tensor("yt", [n_packed, KT, 128, 64], DT.float32,
                              kind="ExternalOutput")
    rem_plan = [] if no_tail else _rem_plan(D, U, M, dp, zero_init)
    n_rem_slots = sum((rem + 31) // 32 for _, rem in rem_plan)
    if n_rem_slots > 0:
        # packed remainder output, transposed: ytr[slot, kt, hcol%128, row]
        ytr_d = nc.dram_tensor("ytr", [n_rem_slots, KT, 128, 32], DT.float32,
                               kind="ExternalOutput")
    if need_zx:
        zx_d = nc.dram_tensor("zx", [max(nzx_rows, 128), G], DT.bfloat16,
                              kind="Internal")
        zx_ap = zx_d.ap()
    hs_d = nc.dram_tensor("hstate", [Ptot, H], DT.bfloat16, kind="Internal")
    cs_d = nc.dram_tensor("cstate", [Ptot, H], DT.float32, kind="Internal")
    if not zero_init:
        ht0_d = nc.dram_tensor("ht0", [KT, 128, M0 * 128], DT.bfloat16,
                               kind="ExternalInput")
        c0_d = nc.dram_tensor("c0", [M0 * 128, H], DT.float32, kind="ExternalInput")

    xt_ap = xt_d.ap(); y_ap = y_d.ap(); hs_ap = hs_d.ap(); cs_ap = cs_d.ap()

    # Waves that keep state SBUF-resident: single m-tile, and the previous
    # wave small enough that its first m-tile's state tiles are still alive
    # (work-pool bufs=3 below).  M is monotone nonincreasing, so once a wave
    # qualifies every later wave does too.
    sbuf_in = [False] * D   # wave d takes h/c from prev wave's SBUF tiles
    for d in range(d_start + 1, min(D, dp)):
        if int(M[d]) == 1 and int(M[d - 1]) <= 3:
            sbuf_in[d] = True
    import os
    if no_tail or os.environ.get("KERNEL_NO_SBUF_TAIL"):
        sbuf_in = [False] * D

    with tile.TileContext(nc) as tc:
        from contextlib import ExitStack
        with ExitStack() as es:
            const = es.enter_context(tc.tile_pool(name="const", bufs=1))
            work = es.enter_context(tc.tile_pool(name="work", bufs=2))
            psum = es.enter_context(tc.tile_pool(name="psum", bufs=1, space="PSUM"))
            whPa = es.enter_context(tc.tile_pool(name="whPa", bufs=1))

            # wave-1's h^T is built incrementally DURING phase A (per depth-0
            # m-tile, straight from the SBUF nhb tile) so phase B's matmuls
            # start without waiting on a 2MB xbar transpose.
            use_hT1 = bool(zero_init and D > 1 and not sbuf_in[1] and dp > 1)
            if use_hT1:
                ncols1 = int(M[1]) * 128
                hT1P = es.enter_context(tc.tile_pool(name="hT1P", bufs=1))
                hT1 = hT1P.tile([128, KT * ncols1], DT.bfloat16, tag="hT1",
                                name="hT1")
                hT1_8 = hT1P.tile([128, KT * ncols1], DT.float8e4,
                                  tag="hT1_8", name="hT1_8")

            ident_sb = const.tile([128, 128], DT.bfloat16, tag="ident")
            # Wh k-slices 0..3 prefetched alongside Wi (fits in SBUF); 4..7
            # loaded once Wi's pool is released, overlapping the first waves.
            # (DMAs for these are emitted a few m-tiles into phase A so the
            # first x/Wi tiles win the DMA queues and the PE starts early.)
            wh_a = whPa.tile([128, 4 * G], DT.float8e4, tag="wha", name="wh_a")
            wh_b = whPa.tile([128, 4 * G], DT.float8e4, tag="whb", name="wh_b")

            def gate_math(z_src, c_src, out_row, store_h, store_c):
                """LSTM gate math for one 128-row m-tile.

                z_src: [128, G] bf16 SBUF tile holding z
                c_src: [128, H] fp32 ap with previous c, or None (c == 0)
                Returns (ncv, nhb) tiles (fp32 cell state, bf16 hidden).
                """
                gi = work.tile([128, H], DT.bfloat16, tag="gi", name="gi", bufs=1)
                gf = work.tile([128, H], DT.bfloat16, tag="gf", name="gf", bufs=1)
                gg = work.tile([128, H], DT.bfloat16, tag="gg", name="gg", bufs=1)
                go = work.tile([128, H], DT.bfloat16, tag="go", name="go", bufs=1)
                nc.scalar.activation(gi[:], z_src[:, 0 * H:1 * H], AF.Sigmoid)
                nc.scalar.activation(gf[:], z_src[:, 1 * H:2 * H], AF.Sigmoid)
                nc.scalar.activation(gg[:], z_src[:, 2 * H:3 * H], AF.Tanh)
                nc.scalar.activation(go[:], z_src[:, 3 * H:4 * H], AF.Sigmoid)

                if c_src is not None:
                    m1 = work.tile([128, H], DT.float32, tag="f32a", name="m1")
                    nc.vector.tensor_mul(m1[:], gi[:], gg[:])
                    t1 = work.tile([128, H], DT.float32, tag="f32b", name="t1")
                    nc.vector.tensor_mul(t1[:], gf[:], c_src[:])
                    ncv = work.tile([128, H], DT.float32, tag="ncv", name="ncv",
                                    bufs=3)
                    nc.vector.tensor_add(ncv[:], t1[:], m1[:])
                else:
                    ncv = work.tile([128, H], DT.float32, tag="ncv", name="ncv",
                                    bufs=3)
                    nc.vector.tensor_mul(ncv[:], gi[:], gg[:])
                tnc = work.tile([128, H], DT.float32, tag="f32b", name="tnc")
                nc.scalar.activation(tnc[:], ncv[:], AF.Tanh)
                nhb = work.tile([128, H], DT.bfloat16, tag="nhb", name="nhb",
                                bufs=3)
                nc.vector.tensor_mul(nhb[:], go[:], tnc[:])

                nc.scalar.dma_start(out=y_ap[out_row:out_row + 128, :],
                                    in_=nhb[:])
                if store_h:
                    nc.scalar.dma_start(out=hs_ap[out_row:out_row + 128, :],
                                        in_=nhb[:])
                if store_c:
                    nc.scalar.dma_start(out=cs_ap[out_row:out_row + 128, :],
                                        in_=ncv[:])
                return ncv, nhb

            def mm_half(lhsT_of_k, rhs_of_k, half, inject=None, k_order=None):
                """Half an m-tile of z accumulation (512 of 1024 cols/gate).

                Half-width PSUM tiles with bufs=2 let consecutive halves /
                m-tiles / waves double-buffer: the next half's matmuls run
                while this half's PSUM is still being drained by DVE/ACT.
                If inject is given (bf16 [128, G] SBUF tile), its half-slices
                are added into PSUM via identity matmuls (z += zx).
                """
                pt = []
                for g in range(4):
                    pt.append(psum.tile([128, 512], DT.float32, tag=f"ph{g}",
                                        name=f"ph{g}", bufs=2))
                ks = list(k_order) if k_order is not None else list(range(KT))
                for i, k in enumerate(ks):
                    lhsT = lhsT_of_k(k)
                    for g in range(4):
                        col0 = g * H + half * 512
                        nc.tensor.matmul(
                            pt[g][:], lhsT=lhsT, rhs=rhs_of_k(k, col0),
                            start=(i == 0),
                            stop=(inject is None and i == len(ks) - 1),
                            skip_group_check=True)
                if inject is not None:
                    for g in range(4):
                        col0 = g * H + half * 512
                        nc.tensor.matmul(
                            pt[g][:], lhsT=ident_sb[:],
                            rhs=inject[:, col0:col0 + 512],
                            start=False, stop=True, skip_group_check=True)
                return pt

            def mm_half_dr(lhsT8_of_kp, rhs8_of_kp, half, inject=None):
                """fp8 DoubleRow half: 4 k-pair matmuls per gate chunk at 2x
                the bf16 PE rate (HW-measured).  kp order mirrors WH_ORDER:
                wh_a's k-slices (4..7 = kp 2,3) stream first."""
                pt = []
                for g in range(4):
                    pt.append(psum.tile([128, 512], DT.float32, tag=f"ph{g}",
                                        name=f"ph{g}", bufs=2))
                kps = (2, 3, 0, 1)
                for i, kp in enumerate(kps):
                    lhsT = lhsT8_of_kp(kp)
                    for g in range(4):
                        col0 = g * H + half * 512
                        nc.tensor.matmul(
                            pt[g][:], lhsT=lhsT, rhs=rhs8_of_kp(kp, col0),
                            start=(i == 0),
                            stop=(inject is None and i == 3),
                            perf_mode=mybir.MatmulPerfMode.DoubleRow,
                            skip_group_check=True)
                if inject is not None:
                    for g in range(4):
                        col0 = g * H + half * 512
                        nc.tensor.matmul(
                            pt[g][:], lhsT=ident_sb[:],
                            rhs=inject[:, col0:col0 + 512],
                            start=False, stop=True, skip_group_check=True)
                return pt

            def cast8(dst_tile, md_, dst_c0, src_view, src_c0, ln):
                """DVE-cast bf16 h^T columns [src_c0, src_c0+ln) (k-major
                4D view [p, kp, j, m]) into an m-tile-major fp8 tile
                [p, (mt kp j m128)] — the LDWEIGHTS-legal DoubleRow lhsT
                layout (pair stride 128, row stride 1)."""
                v8 = dst_tile[:, 0:md_ * KT * 128].rearrange(
                    "p (mt kp j m) -> p mt kp j m",
                    mt=md_, kp=KT // 2, j=2, m=128)
                off = 0
                while off < ln:
                    dmt = (dst_c0 + off) // 128
                    dof = (dst_c0 + off) % 128
                    step = min(ln - off, 128 - dof)
                    nc.vector.tensor_copy(
                        v8[:, dmt, :, :, dof:dof + step],
                        src_view[:, :, :, src_c0 + off:src_c0 + off + step])
                    off += step

            def kmajor_view(t, ncols_):
                return t[:, 0:KT * ncols_].rearrange(
                    "p (kp j m) -> p kp j m", kp=KT // 2, j=2, m=ncols_)

            # ---------------- phase A: zx = x@Wi + bias ----------------
            with tc.tile_pool(name="wiP", bufs=1) as wiP, \
                 tc.tile_pool(name="xtP", bufs=3) as xtP:
                wi_sb = wiP.tile([128, KT * GBF], DT.bfloat16, tag="wi",
                                 name="wi_sb")
                if f8_gates:
                    wi8_sb = wiP.tile([128, KT * GF8], DT.float8e4, tag="wi8",
                                      name="wi8_sb")
                    wi8_3 = wi8_sb.rearrange("p (j n) -> p j n", n=GF8)
                # emission order = DMA priority: k=0 and the first x tile win
                # the queues so the PE can start ~5us in
                xt0_sb = xtP.tile([128, CIN], DT.bfloat16, tag="xt", name="xt_sb")
                nc.gpsimd.dma_start(out=xt0_sb[:, 0:128], in_=xt_ap[0][:, 0:128])
                for ch in range(GBF // 1024):
                    nc.sync.dma_start(
                        out=wi_sb[:, ch * 1024:(ch + 1) * 1024],
                        in_=wi_d.ap()[0][:, ch * 1024:(ch + 1) * 1024])
                nc.gpsimd.dma_start(out=xt0_sb[:, 128:CIN],
                                    in_=xt_ap[0][:, 128:CIN])
                bbc_sb = wiP.tile([128, G], DT.bfloat16, tag="bbc", name="bbc_sb")
                nc.sync.dma_start(out=bbc_sb[:], in_=bbc_d.ap()[:])
                nc.sync.dma_start(out=ident_sb[:], in_=id_d.ap()[:])
                if f8_gates:
                    xt80_sb = xtP.tile([128, CIN], DT.float8e4, tag="xt8",
                                       name="xt8_sb")
                    nc.sync.dma_start(out=xt80_sb[:], in_=xt8_d.ap()[0])
                    nc.sync.dma_start(out=wi8_sb[:, 0:GF8], in_=wi8_d.ap()[0])
                    nc.sync.dma_start(out=wi8_sb[:, GF8:2 * GF8],
                                      in_=wi8_d.ap()[1])
                # Wi k>=1 rides the sync/scalar queues so the per-m-tile xt
                # stream (gpsimd) never queues behind megabytes of weights
                for k in range(1, KT):
                    nc.sync.dma_start(out=wi_sb[:, k * GBF:(k + 1) * GBF],
                                      in_=wi_d.ap()[k])
                    if f8_gates and k >= 2:
                        nc.scalar.dma_start(
                            out=wi8_sb[:, k * GF8:(k + 1) * GF8],
                            in_=wi8_d.ap()[k])

                for mt in range(Mzx):
                    if mt == min(3, Mzx - 1) and (need_zx or D > 1):
                        # prefetch Wh (fp8, 4MB total) k=4..7 then k=0..3
                        # (scalar queue: y stores are slack, xt must not wait)
                        for k in range(4):
                            nc.scalar.dma_start(out=wh_a[:, k * G:(k + 1) * G],
                                                in_=wh_d.ap()[4 + k])
                    if mt == min(6, Mzx - 1) and (need_zx or D > 1) \
                            and Mzx > 3:
                        for k in range(4):
                            nc.scalar.dma_start(out=wh_b[:, k * G:(k + 1) * G],
                                                in_=wh_d.ap()[k])
                    if mt == 0:
                        xt_sb = xt0_sb
                        xt8_sb = xt80_sb if f8_gates else None
                    else:
                        xt_sb = xtP.tile([128, CIN], DT.bfloat16, tag="xt",
                                         name="xt_sb")
                        nc.gpsimd.dma_start(out=xt_sb[:], in_=xt_ap[mt])
                        if f8_gates:
                            xt8_sb = xtP.tile([128, CIN], DT.float8e4,
                                              tag="xt8", name="xt8_sb")
                            nc.gpsimd.dma_start(out=xt8_sb[:],
                                                in_=xt8_d.ap()[mt])

                    zxt = work.tile([128, G], DT.bfloat16, tag="zws", name="zxt")
                    for half in range(2):
                        pt = []
                        for g in range(4):
                            pt.append(psum.tile([128, 512], DT.float32,
                                                tag=f"ph{g}", name=f"ph{g}",
                                                bufs=2))
                        for k in range(KT):
                            lhsT = xt_sb[:, k * 128:(k + 1) * 128]
                            for gb, g in enumerate(bf_gates):
                                col0 = gb * H + half * 512
                                nc.tensor.matmul(
                                    pt[g][:], lhsT=lhsT,
                                    rhs=wi_sb[:, k * GBF + col0:
                                              k * GBF + col0 + 512],
                                    start=(k == 0), stop=(k == KT - 1),
                                    skip_group_check=True)
                        for kp in range(KT // 2):
                            if not f8_gates:
                                break
                            lhsT8 = xt8_sb[:, kp * 256:(kp + 1) * 256] \
                                .rearrange("p (j m) -> p j m", m=128)
                            for g8, g in enumerate(f8_gates):
                                col0 = g8 * H + half * 512
                                nc.tensor.matmul(
                                    pt[g][:], lhsT=lhsT8,
                                    rhs=wi8_3[:, 2 * kp:2 * kp + 2,
                                              col0:col0 + 512],
                                    start=(kp == 0), stop=(kp == KT // 2 - 1),
                                    perf_mode=mybir.MatmulPerfMode.DoubleRow,
                                    skip_group_check=True)
                        # psum -> SBUF with bias add (bf16)
                        for g in range(4):
                            col0 = g * H + half * 512
                            nc.vector.tensor_add(zxt[:, col0:col0 + 512],
                                                 pt[g][:],
                                                 bbc_sb[:, col0:col0 + 512])

                    if need_zx and mt >= zx_start_tile:
                        r = mt * 128 - zx_row0
                        nc.gpsimd.dma_start(out=zx_ap[r:r + 128, :], in_=zxt[:])
                    if zero_init and mt < M0:
                        nwave1 = int(M[1]) if D > 1 else 0
                        ncv0, nhb0 = gate_math(
                            zxt, None, mt * 128,
                            store_h=(D > 1 and not sbuf_in[1]
                                     and not use_hT1 and mt < nwave1),
                            store_c=(D > 1 and not sbuf_in[1]
                                     and mt < nwave1))
                        if use_hT1 and mt < nwave1:
                            nc.sync.dma_start_transpose(
                                out=hT1.rearrange(
                                    "p (j m) -> p j m",
                                    m=ncols1)[:, :, mt * 128:(mt + 1) * 128],
                                in_=nhb0[:])
                            cast8(hT1_8, int(M[1]), mt * 128,
                                  kmajor_view(hT1, ncols1), mt * 128, 128)

            # ---------------- phase B: waves ----------------
            if D > d_start:
                with tc.tile_pool(name="hTP", bufs=2) as hTP, \
                     tc.tile_pool(name="hT8P", bufs=2) as hT8P, \
                     tc.tile_pool(name="zxP", bufs=2) as zxP, \
                     tc.tile_pool(name="cP", bufs=2) as cP:
                    WH_ORDER = [4, 5, 6, 7, 0, 1, 2, 3]

                    def wh_rhs(k, col0):
                        if k >= 4:
                            return wh_a[:, (k - 4) * G + col0:
                                        (k - 4) * G + col0 + 512]
                        return wh_b[:, k * G + col0:k * G + col0 + 512]

                    # DoubleRow fp8 rhs: k-tile pair kp -> [128, 2, 512]
                    wh_a3 = wh_a.rearrange("p (j n) -> p j n", n=G)
                    wh_b3 = wh_b.rearrange("p (j n) -> p j n", n=G)

                    def wh8_rhs(kp, col0):
                        if kp >= 2:
                            return wh_a3[:, 2 * (kp - 2):2 * (kp - 2) + 2,
                                         col0:col0 + 512]
                        return wh_b3[:, 2 * kp:2 * kp + 2, col0:col0 + 512]

                    cand = [KT * 128]
                    for d2 in range(max(d_start, 1), min(D, dp)):
                        if sbuf_in[d2] or (d2 == 1 and use_hT1):
                            continue
                        cand.append(KT * int(M[d2]) * 128)
                    hT_cols_max = max(cand)
                    prev_ncv = None
                    prev_nhb = None
                    pending_hT = None  # hT pre-built by a tail wave for d+1
                    pending_hT8 = None
                    prebuilt_hT = None  # hT pre-built by a DRAM wave for d+1
                    prebuilt_hT8 = None
                    rem_of = dict(rem_plan)
                    rem_slot = [0]
                    if n_rem_slots > 0 or n_packed > 0:
                        identf = const.tile([128, 128], DT.float32, tag="idf",
                                            name="identf")
                        nc.gpsimd.dma_start(out=identf[:], in_=idf_d.ap()[:])
                    if n_rem_slots > 0:
                        ytr_ap = ytr_d.ap()

                    # tail zx prefetch: every tail pass's zx strips are pulled
                    # into SBUF during the waves (a few slots per wave m-tile
                    # on the gpsimd queue), so no tail pass waits on DRAM
                    tail_slots = []
                    tail_pf = [0]
                    if n_packed > 0:
                        for d_ in range(dp, D):
                            for p_ in range((int(U[d_]) + 31) // 32):
                                for h_ in (1, 0):
                                    tail_slots.append((d_, p_, h_))
                        zxtl = zxP.tile([128, len(tail_slots) * 512],
                                        DT.bfloat16, tag="zxtl", name="zxtl",
                                        bufs=1)
                        tail_slot_of = {key: s for s, key in
                                        enumerate(tail_slots)}

                    def emit_tail_pf(nslots):
                        while tail_pf[0] < len(tail_slots) and nslots > 0:
                            s = tail_pf[0]
                            d_, p_, h_ = tail_slots[s]
                            rz = int(V[d_]) - zx_row0 + 32 * p_
                            for g in range(4):
                                nc.gpsimd.dma_start(
                                    out=zxtl[32 * g:32 * g + 32,
                                             s * 512:(s + 1) * 512],
                                    in_=zx_ap[rz:rz + 32,
                                              g * H + 512 * h_:
                                              g * H + 512 * h_ + 512])
                            tail_pf[0] += 1
                            nslots -= 1

                    def packed_rem(d, rem, hT_buf, hT_ncols):
                        """Run a wave's last-tile remainder (<=64 live rows,
                        y-only consumers) through the packed col-tiled
                        pipeline: ~3.4us per 32 rows vs 13.8us full-width."""
                        Md = int(M[d])
                        rb = (Md - 1) * 128
                        rem_pad = ((rem + 31) // 32) * 32
                        prevP = int(P[d - 1])
                        ctmp = cP.tile([128, H], DT.float32, tag="c",
                                       name="ctmp")
                        nc.gpsimd.dma_start(
                            out=ctmp[0:rem_pad, :],
                            in_=cs_ap[prevP + rb:prevP + rb + rem_pad, :])
                        cT_rem = cP.tile([128, KT, 64], DT.float32, tag="c",
                                         name="cT_rem")
                        for kt in range(KT):
                            ph_c = psum.tile([128, 64], DT.float32, tag="ph1",
                                             name="ph_c", bufs=2)
                            nc.tensor.transpose(
                                ph_c[:, 0:rem_pad],
                                ctmp[0:rem_pad, kt * 128:(kt + 1) * 128],
                                identf[0:rem_pad, 0:rem_pad])
                            nc.vector.tensor_copy(cT_rem[:, kt, 0:rem_pad],
                                                  ph_c[:, 0:rem_pad])
                        for p in range((rem + 31) // 32):
                            r0p = 32 * p
                            slot = rem_slot[0]
                            rem_slot[0] += 1
                            for h in (1, 0):
                                zx_pk = zxP.tile([128, 512], DT.bfloat16,
                                                 tag="zx", name="zx_pkr",
                                                 bufs=2)
                                rz = int(V[d]) - zx_row0 + rb + r0p
                                for g in range(4):
                                    nc.gpsimd.dma_start(
                                        out=zx_pk[32 * g:32 * g + 32, :],
                                        in_=zx_ap[rz:rz + 32,
                                                  g * H + 512 * h:
                                                  g * H + 512 * h + 512])
                                pz = psum.tile([128, 512], DT.float32,
                                               tag="ph2", name="pzr", bufs=2)
                                for i, k in enumerate(range(KT)):
                                    lt = hT_buf[:, k * hT_ncols + rb + r0p:
                                                k * hT_ncols + rb + r0p + 32]
                                    for g in range(4):
                                        nc.tensor.matmul(
                                            pz[32 * g:32 * g + 32, :],
                                            lhsT=lt,
                                            rhs=wh_rhs(k, g * H + 512 * h),
                                            start=(i == 0),
                                            stop=(i == KT - 1),
                                            tile_position=(0, 32 * g),
                                            skip_group_check=True)
                                z_pk = work.tile([128, 512], DT.bfloat16,
                                                 tag="zws", name="z_pkr",
                                                 bufs=2)
                                nc.vector.tensor_add(z_pk[:], pz[:], zx_pk[:])
                                zt = psum.tile([128, 512], DT.bfloat16,
                                               tag="ph3", name="ztr", bufs=2)
                                for c in range(4):
                                    nc.tensor.transpose(
                                        zt[:, 128 * c:128 * c + 128],
                                        z_pk[:, 128 * c:128 * c + 128],
                                        ident_sb[:])
                                ztv = zt.rearrange("p (c g r) -> p c g r",
                                                   c=4, g=4)
                                gt = work.tile([128, 4, 4, 32], DT.bfloat16,
                                               tag="m1b", name="gtr", bufs=2)
                                nc.scalar.activation(gt[:, :, 0:2, :],
                                                     ztv[:, :, 0:2, :],
                                                     AF.Sigmoid)
                                nc.scalar.activation(gt[:, :, 2, :],
                                                     ztv[:, :, 2, :], AF.Tanh)
                                nc.scalar.activation(gt[:, :, 3, :],
                                                     ztv[:, :, 3, :],
                                                     AF.Sigmoid)
                                m1p = work.tile([128, 4, 32], DT.float32,
                                                tag="t1b", name="m1pr",
                                                bufs=2)
                                nc.vector.tensor_mul(m1p[:], gt[:, :, 0, :],
                                                     gt[:, :, 2, :])
                                t1p = work.tile([128, 4, 32], DT.float32,
                                                tag="tncb", name="t1pr",
                                                bufs=2)
                                nc.vector.tensor_mul(
                                    t1p[:], gt[:, :, 1, :],
                                    cT_rem[:, 4 * h:4 * h + 4,
                                           r0p:r0p + 32])
                                ncsr = work.tile([128, 4, 32], DT.float32,
                                                 tag="ncsr", name="ncsr",
                                                 bufs=2)
                                nc.vector.tensor_add(ncsr[:], t1p[:], m1p[:])
                                tncp = work.tile([128, 4, 32], DT.float32,
                                                 tag="ptnc", name="tncpr",
                                                 bufs=2)
                                nc.scalar.activation(tncp[:], ncsr[:],
                                                     AF.Tanh)
                                nhfp = work.tile([128, 4, 32], DT.float32,
                                                 tag="pnh", name="nhfpr",
                                                 bufs=2)
                                nc.vector.tensor_mul(nhfp[:], gt[:, :, 3, :],
                                                     tncp[:])
                                for c_ in range(4):
                                    nc.sync.dma_start(
                                        out=ytr_ap[slot, 4 * h + c_, :, :],
                                        in_=nhfp[:, c_, :])

                    for d in range(d_start, min(D, dp)):
                        Md = int(M[d])
                        ncols = Md * 128

                        if sbuf_in[d]:
                            # ---- SBUF-resident tail wave (one m-tile) ----
                            zx_sb = zxP.tile([128, G], DT.bfloat16, tag="zx",
                                             name="zx_sb")
                            r = int(V[d]) - zx_row0
                            nc.gpsimd.dma_start(out=zx_sb[:],
                                                in_=zx_ap[r:r + 128, :])

                            import os as _os2
                            _no_xbar = bool(
                                _os2.environ.get("KERNEL_TAIL_NO_XBAR"))
                            if pending_hT is not None:
                                hT = pending_hT
                                hT8s = pending_hT8
                            else:
                                hT = hTP.tile([128, KT * 128], DT.bfloat16,
                                              tag="hTs", name="hTs")
                                if _no_xbar:
                                    for kk in range(KT):
                                        bk = slice(kk * 128, (kk + 1) * 128)
                                        nc.sync.dma_start(
                                            out=hT[:, bk],
                                            in_=prev_nhb[:, bk].rearrange(
                                                "a b -> b a"))
                                else:
                                    nc.sync.dma_start_transpose(
                                        out=hT.rearrange("p (j c) -> p j c",
                                                         c=128),
                                        in_=prev_nhb[:])
                                hT8s = hT8P.tile([128, KT * 128],
                                                 DT.float8e4, tag="hT8s",
                                                 name="hT8s")
                                cast8(hT8s, 1, 0, kmajor_view(hT, 128),
                                      0, 128)

                            gi = work.tile([128, H], DT.bfloat16, tag="gi",
                                           name="gi", bufs=1)
                            gf = work.tile([128, H], DT.bfloat16, tag="gf",
                                           name="gf", bufs=1)
                            gg = work.tile([128, H], DT.bfloat16, tag="gg",
                                           name="gg", bufs=1)
                            go = work.tile([128, H], DT.bfloat16, tag="go",
                                           name="go", bufs=1)
                            ncv = work.tile([128, H], DT.float32, tag="ncv",
                                            name="ncv", bufs=3)
                            nhb = work.tile([128, H], DT.bfloat16, tag="nhb",
                                            name="nhb", bufs=3)
                            build_next = d + 1 < min(D, dp)
                            if build_next:
                                hT_next = hTP.tile([128, KT * 128], DT.bfloat16,
                                                   tag="hTs", name="hTn")
                                hT8_next = hT8P.tile([128, KT * 128],
                                                     DT.float8e4, tag="hT8s",
                                                     name="hT8n")
                            # half-pass z accumulation + 256-col-block gate
                            # chains: each block's h^T transpose is issued as
                            # soon as its nhb quarter is ready, so the next
                            # wave's matmuls overlap this wave's gate math
                            hT8s_v = hT8s[:].rearrange(
                                "p (kp j m) -> p kp j m", kp=KT // 2, j=2,
                                m=128)
                            lhsT8_of_kp = (lambda hv: lambda kp:
                                           hv[:, kp])(hT8s_v)
                            # half 1 first: the next wave's matmuls consume
                            # k=4..7 (kp 2,3) before k=0..3, so produce the
                            # matching h^T blocks first
                            for half in (1, 0):
                                pt = mm_half_dr(lhsT8_of_kp, wh8_rhs, half,
                                                inject=zx_sb)
                                blk = slice(half * 512, half * 512 + 512)
                                nc.scalar.activation(gi[:, blk], pt[0][:],
                                                     AF.Sigmoid)
                                nc.scalar.activation(gg[:, blk], pt[2][:],
                                                     AF.Tanh)
                                m1b = work.tile([128, 512], DT.float32,
                                                tag="m1b", name="m1b", bufs=2)
                                nc.vector.tensor_mul(m1b[:], gi[:, blk],
                                                     gg[:, blk])
                                nc.scalar.activation(gf[:, blk], pt[1][:],
                                                     AF.Sigmoid)
                                t1b = work.tile([128, 512], DT.float32,
                                                tag="t1b", name="t1b", bufs=2)
                                nc.vector.tensor_mul(t1b[:], gf[:, blk],
                                                     prev_ncv[:, blk])
                                nc.vector.tensor_add(ncv[:, blk], t1b[:],
                                                     m1b[:])
                                nc.scalar.activation(go[:, blk], pt[3][:],
                                                     AF.Sigmoid)
                                tncb = work.tile([128, 512], DT.float32,
                                                 tag="tncb", name="tncb",
                                                 bufs=2)
                                nc.scalar.activation(tncb[:], ncv[:, blk],
                                                     AF.Tanh)
                                nc.vector.tensor_mul(nhb[:, blk], go[:, blk],
                                                     tncb[:])
                                if build_next:
                                    if _no_xbar:
                                        for kk in range(half * 4, half * 4 + 4):
                                            bk = slice(kk * 128, (kk + 1) * 128)
                                            nc.sync.dma_start(
                                                out=hT_next[:, bk],
                                                in_=nhb[:, bk].rearrange(
                                                    "a b -> b a"))
                                    else:
                                        # blocked transpose of this half's 4
                                        # k-chunks in ONE xbar DMA
                                        nc.sync.dma_start_transpose(
                                            out=hT_next.rearrange(
                                                "p (j c) -> p j c", c=128)
                                            [:, half * 4:half * 4 + 4, :],
                                            in_=nhb[:, blk])
                                    # fp8 mirror of this half's kp chunks
                                    hT8n_v = hT8_next[:].rearrange(
                                        "p (kp j m) -> p kp j m",
                                        kp=KT // 2, j=2, m=128)
                                    nc.vector.tensor_copy(
                                        hT8n_v[:, 2 * half:2 * half + 2],
                                        kmajor_view(hT_next, 128)
                                        [:, 2 * half:2 * half + 2])

                            nc.scalar.dma_start(
                                out=y_ap[int(P[d]):int(P[d]) + 128, :],
                                in_=nhb[:])
                            prev_ncv, prev_nhb = ncv, nhb
                            pending_hT = hT_next if build_next else None
                            pending_hT8 = hT8_next if build_next else None
                            continue

                        # ---- DRAM-path wave ----
                        pending_hT = None
                        pending_hT8 = None
                        if d == 0:
                            hT = None  # allocated per m-tile below
                            hT8 = None
                        elif d == 1 and use_hT1:
                            hT = hT1  # built during phase A
                            hT8 = hT1_8
                        elif prebuilt_hT is not None:
                            hT = prebuilt_hT  # built during previous wave
                            hT8 = prebuilt_hT8
                        else:
                            hT = hTP.tile([128, hT_cols_max], DT.bfloat16,
                                          tag="hT", name="hT")
                            prev = int(P[d - 1])
                            nc.sync.dma_start_transpose(
                                out=hT[:, 0:KT * ncols].rearrange(
                                    "p (j m) -> p j m", m=ncols),
                                in_=hs_ap[prev:prev + ncols, :])
                            hT8 = hT8P.tile([128, hT_cols_max], DT.float8e4,
                                            tag="hT8", name="hT8")
                            cast8(hT8, Md, 0, kmajor_view(hT, ncols),
                                  0, ncols)
                        if d == 2 and merge_rem > 0:
                            # merged wave-1 rows: their input h^T is the
                            # depth-0 output, copied straight out of hT1
                            src0 = (int(M[1]) - 1) * 128
                            ext0_2 = ((int(U[2]) + 15) // 16) * 16
                            nc.vector.tensor_copy(
                                hT[:, 0:KT * ncols].rearrange(
                                    "p (j m) -> p j m", m=ncols)
                                [:, :, ext0_2:ext0_2 + merge_rem],
                                hT1.rearrange(
                                    "p (j m) -> p j m", m=int(M[1]) * 128)
                                [:, :, src0:src0 + merge_rem])
                            cast8(hT8, Md, ext0_2,
                                  kmajor_view(hT1, int(M[1]) * 128),
                                  src0, merge_rem)
                        prebuilt_hT = None
                        prebuilt_hT8 = None
                        nxt_dram_hT = None
                        nxt_dram_hT8 = None
                        nxt_ncols = 0
                        if d + 1 < min(D, dp) and not sbuf_in[d + 1]:
                            nxt_ncols = int(M[d + 1]) * 128
                            nxt_dram_hT = hTP.tile([128, hT_cols_max],
                                                   DT.bfloat16, tag="hT",
                                                   name="hTn")
                            nxt_dram_hT8 = hT8P.tile([128, hT_cols_max],
                                                     DT.float8e4, tag="hT8",
                                                     name="hT8n")
                        hT8v = None
                        if d > 0:
                            hT8v = hT8[:, 0:Md * KT * 128].rearrange(
                                "p (mt kp j m) -> p mt kp j m", mt=Md,
                                kp=KT // 2, j=2, m=128)

                        n_full = Md - 1 if d in rem_of else Md
                        if d == 1 and merge_rem > 0:
                            n_full = Md - 1
                        for mt in range(n_full):
                            zx_sb = zxP.tile([128, G], DT.bfloat16, tag="zx",
                                             name="zx_sb")
                            r = int(V[d]) + mt * 128 - zx_row0
                            nc.gpsimd.dma_start(out=zx_sb[:],
                                                in_=zx_ap[r:r + 128, :])

                            if d == 0:
                                c_src = cP.tile([128, H], DT.float32, tag="c",
                                                name="c_sb")
                                nc.gpsimd.dma_start(
                                    out=c_src[:],
                                    in_=c0_d.ap()[mt * 128:mt * 128 + 128, :])
                            else:
                                c_src = cP.tile([128, H], DT.float32, tag="c",
                                                name="c_sb")
                                prev = int(P[d - 1])
                                nc.gpsimd.dma_start(
                                    out=c_src[:],
                                    in_=cs_ap[prev + mt * 128:
                                              prev + mt * 128 + 128, :])

                            if d >= 1:
                                emit_tail_pf(3)
                            if d == 2 and merge_rem > 0 and mt == Md - 1:
                                # merged wave-1 rows: overwrite their zx and
                                # c slots (their h^T was copied from hT1)
                                loc0 = ((int(U[2]) + 15) // 16) * 16 \
                                    - mt * 128
                                src0 = (int(M[1]) - 1) * 128
                                nc.gpsimd.dma_start(
                                    out=zx_sb[loc0:loc0 + merge_rem, :],
                                    in_=zx_ap[int(V[1]) + src0 - zx_row0:
                                              int(V[1]) + src0 - zx_row0
                                              + merge_rem, :])
                                nc.gpsimd.dma_start(
                                    out=c_src[loc0:loc0 + merge_rem, :],
                                    in_=cs_ap[src0:src0 + merge_rem, :])
                            if d == 0:
                                hT = hTP.tile([128, KT * 128], DT.bfloat16,
                                              tag="hT0", name="hT0")
                                for k in range(KT):
                                    nc.gpsimd.dma_start(
                                        out=hT[:, k * 128:(k + 1) * 128],
                                        in_=ht0_d.ap()[k][:, mt * 128:
                                                          mt * 128 + 128])
                                lhsT_of_k = (lambda hh: lambda k:
                                             hh[:, k * 128:(k + 1) * 128])(hT)
                            else:
                                lhsT8_of_kp = (lambda hv, mm: lambda kp:
                                               hv[:, mm, kp])(hT8v, mt)

                            nxt_sbuf = (d + 1 < D) and sbuf_in[d + 1]
                            nwave = int(M[d + 1]) * 128 if d + 1 < D else 0
                            out_row = int(P[d]) + mt * 128
                            store_c = (d + 1 < D and not nxt_sbuf
                                       and mt * 128 < nwave)
                            if d == 0:
                                z_sb = work.tile([128, G], DT.bfloat16,
                                                 tag="zws", name="z_sb")
                                for half in range(2):
                                    pt = mm_half(lhsT_of_k, wh_rhs, half,
                                                 k_order=WH_ORDER)
                                    for g in range(4):
                                        col0 = g * H + half * 512
                                        nc.vector.tensor_add(
                                            z_sb[:, col0:col0 + 512],
                                            pt[g][:],
                                            zx_sb[:, col0:col0 + 512])
                                ncv, nhb = gate_math(z_sb, c_src, out_row,
                                                     store_h=False,
                                                     store_c=store_c)
                            else:
                                # PSUM-direct gate math: zx is folded into
                                # PSUM by the PE inject, so DVE runs only the
                                # 4-op state chain (phase B is DVE-bound)
                                gi = work.tile([128, H], DT.bfloat16,
                                               tag="gi", name="gi", bufs=1)
                                gf = work.tile([128, H], DT.bfloat16,
                                               tag="gf", name="gf", bufs=1)
                                gg = work.tile([128, H], DT.bfloat16,
                                               tag="gg", name="gg", bufs=1)
                                go = work.tile([128, H], DT.bfloat16,
                                               tag="go", name="go", bufs=1)
                                ncv = work.tile([128, H], DT.float32,
                                                tag="ncv", name="ncv", bufs=3)
                                nhb = work.tile([128, H], DT.bfloat16,
                                                tag="nhb", name="nhb", bufs=3)
                                for half in (1, 0):
                                    pt = mm_half_dr(lhsT8_of_kp, wh8_rhs,
                                                    half, inject=zx_sb)
                                    blk = slice(half * 512, half * 512 + 512)
                                    nc.scalar.activation(gi[:, blk], pt[0][:],
                                                         AF.Sigmoid)
                                    nc.scalar.activation(gg[:, blk], pt[2][:],
                                                         AF.Tanh)
                                    m1b = work.tile([128, 512], DT.float32,
                                                    tag="m1b", name="m1b",
                                                    bufs=2)
                                    nc.vector.tensor_mul(m1b[:], gi[:, blk],
                                                         gg[:, blk])
                                    nc.scalar.activation(gf[:, blk], pt[1][:],
                                                         AF.Sigmoid)
                                    t1b = work.tile([128, 512], DT.float32,
                                                    tag="t1b", name="t1b",
                                                    bufs=2)
                                    nc.vector.tensor_mul(t1b[:], gf[:, blk],
                                                         c_src[:, blk])
                                    nc.vector.tensor_add(ncv[:, blk], t1b[:],
                                                         m1b[:])
                                    nc.scalar.activation(go[:, blk], pt[3][:],
                                                         AF.Sigmoid)
                                    tncb = work.tile([128, 512], DT.float32,
                                                     tag="tncb", name="tncb",
                                                     bufs=2)
                                    nc.scalar.activation(tncb[:], ncv[:, blk],
                                                         AF.Tanh)
                                    nc.vector.tensor_mul(nhb[:, blk],
                                                         go[:, blk], tncb[:])
                                nc.scalar.dma_start(
                                    out=y_ap[out_row:out_row + 128, :],
                                    in_=nhb[:])
                                if store_c:
                                    nc.scalar.dma_start(
                                        out=cs_ap[out_row:out_row + 128, :],
                                        in_=ncv[:])
                            if mt == 0:
                                prev_ncv, prev_nhb = ncv, nhb
                            if mt == 0 and d in rem_of:
                                packed_rem(d, rem_of[d], hT, ncols)
                            if nxt_dram_hT is not None \
                                    and mt * 128 < nxt_ncols:
                                # when merged rows will be copied into the
                                # padding columns, keep this transpose's
                                # write region disjoint from the copy's
                                hi = (mt + 1) * 128
                                if d == 1 and merge_rem > 0:
                                    hi = min(hi,
                                             ((int(U[2]) + 15) // 16) * 16)
                                nc.sync.dma_start_transpose(
                                    out=nxt_dram_hT[:, 0:KT * nxt_ncols]
                                    .rearrange("p (j m) -> p j m",
                                               m=nxt_ncols)
                                    [:, :, mt * 128:hi],
                                    in_=nhb[0:hi - mt * 128, :])
                                cast8(nxt_dram_hT8, int(M[d + 1]), mt * 128,
                                      kmajor_view(nxt_dram_hT, nxt_ncols),
                                      mt * 128, hi - mt * 128)
                            elif nxt_sbuf and mt == 0:
                                hTs_n = hTP.tile([128, KT * 128],
                                                 DT.bfloat16, tag="hTs",
                                                 name="hTs_n")
                                nc.sync.dma_start_transpose(
                                    out=hTs_n.rearrange("p (j c) -> p j c",
                                                        c=128),
                                    in_=nhb[:])
                                hTs_n8 = hT8P.tile([128, KT * 128],
                                                   DT.float8e4, tag="hT8s",
                                                   name="hTs_n8")
                                cast8(hTs_n8, 1, 0, kmajor_view(hTs_n, 128),
                                      0, 128)
                                pending_hT = hTs_n
                                pending_hT8 = hTs_n8
                        prebuilt_hT = nxt_dram_hT
                        prebuilt_hT8 = nxt_dram_hT8

                    # -------- packed col-tiled tail (waves dp..D-1) --------
                    # Each wave has <=64 live rows.  The 4 gates' z chunks are
                    # computed CONCURRENTLY on the PE's four 32-wide column
                    # strips (tile_position col tiling, ~4x the Wh streaming
                    # rate), packed into one PSUM tile [4*32, 512].  zx is
                    # folded in by the PSUM->SBUF move, then PE transposes put
                    # h-columns on partitions so the tiny row count is the
                    # free dim: gate math runs on [128, 4, 32] strided views
                    # and its output nhT IS the next wave's stationary
                    # operand - no DMA transposes, no DRAM round trips.
                    if n_packed > 0:
                        yt_ap = yt_d.ap()
                        KORD = [4, 5, 6, 7, 0, 1, 2, 3]
                        rows0 = min(64, ((int(U[dp]) + 31) // 32) * 32)

                        # handoff: transpose prev wave's nh/c row prefix into
                        # [hcol-partition, row-free] layout
                        if sbuf_in[dp - 1]:
                            src_h, src_c = prev_nhb, prev_ncv
                        else:
                            src_h = hTP.tile([128, H], DT.bfloat16, tag="hoh",
                                             name="src_h")
                            src_c = cP.tile([128, H], DT.float32, tag="hoc",
                                            name="src_c")
                            prevr = int(P[dp - 1])
                            nc.gpsimd.dma_start(
                                out=src_h[:], in_=hs_ap[prevr:prevr + 128, :])
                            nc.gpsimd.dma_start(
                                out=src_c[:], in_=cs_ap[prevr:prevr + 128, :])
                        nhT_prev = hTP.tile([128, KT, 64], DT.float8e4,
                                            tag="nhT", name="nhT0", bufs=2)
                        cT_prev = cP.tile([128, KT, 64], DT.float32,
                                          tag="c", name="cT0", bufs=2)
                        for kt in KORD:
                            ph_h = psum.tile([128, 64], DT.bfloat16, tag="ph0",
                                             name="ph_h", bufs=2)
                            nc.tensor.transpose(
                                ph_h[:, 0:rows0],
                                src_h[0:rows0, kt * 128:(kt + 1) * 128],
                                ident_sb[0:rows0, 0:rows0])
                            nc.vector.tensor_copy(nhT_prev[:, kt, 0:rows0],
                                                  ph_h[:, 0:rows0])
                            ph_c = psum.tile([128, 64], DT.float32, tag="ph1",
                                             name="ph_c", bufs=2)
                            nc.tensor.transpose(
                                ph_c[:, 0:rows0],
                                src_c[0:rows0, kt * 128:(kt + 1) * 128],
                                identf[0:rows0, 0:rows0])
                            nc.vector.tensor_copy(cT_prev[:, kt, 0:rows0],
                                                  ph_c[:, 0:rows0])

                        emit_tail_pf(len(tail_slots))

                        for w, d in enumerate(range(dp, D)):
                            rows_d = int(U[d])
                            npass = (rows_d + 31) // 32
                            nhT_new = hTP.tile([128, KT, 64], DT.float8e4,
                                               tag="nhT", name="nhT_n", bufs=2)
                            cT_new = cP.tile([128, KT, 64], DT.float32,
                                             tag="c", name="cT_n", bufs=2)
                            for p in range(npass):
                                r0 = 32 * p
                                for ri, h in enumerate((1, 0)):
                                    ptag = ["ph2", "ph0"][(2 * p + ri) % 2]
                                    ttag = ["ph3", "ph1"][(2 * p + ri) % 2]
                                    sl = tail_slot_of[(d, p, h)]
                                    zx_pk = zxtl[:, sl * 512:(sl + 1) * 512]
                                    pz = psum.tile([128, 512], DT.float32,
                                                   tag=ptag, name="pz", bufs=2)
                                    for i, k in enumerate(KORD):
                                        lt = nhT_prev[:, k, r0:r0 + 32]
                                        for g in range(4):
                                            nc.tensor.matmul(
                                                pz[32 * g:32 * g + 32, :],
                                                lhsT=lt,
                                                rhs=wh_rhs(k, g * H + 512 * h),
                                                start=(i == 0),
                                                stop=(i == KT - 1),
                                                tile_position=(0, 32 * g),
                                                skip_group_check=True)
                                    z_pk = work.tile([128, 512], DT.bfloat16,
                                                     tag="zws", name="z_pk",
                                                     bufs=2)
                                    nc.vector.tensor_add(z_pk[:], pz[:],
                                                         zx_pk[:])
                                    zt = psum.tile([128, 512], DT.bfloat16,
                                                   tag=ttag, name="zt", bufs=2)
                                    for c in range(4):
                                        nc.tensor.transpose(
                                            zt[:, 128 * c:128 * c + 128],
                                            z_pk[:, 128 * c:128 * c + 128],
                                            ident_sb[:])
                                    ztv = zt.rearrange("p (c g r) -> p c g r",
                                                       c=4, g=4)
                                    gt = work.tile([128, 4, 4, 32], DT.bfloat16,
                                                   tag="m1b", name="gt", bufs=2)
                                    nc.scalar.activation(gt[:, :, 0:2, :],
                                                         ztv[:, :, 0:2, :],
                                                         AF.Sigmoid)
                                    nc.scalar.activation(gt[:, :, 2, :],
                                                         ztv[:, :, 2, :],
                                                         AF.Tanh)
                                    nc.scalar.activation(gt[:, :, 3, :],
                                                         ztv[:, :, 3, :],
                                                         AF.Sigmoid)
                                    m1p = work.tile([128, 4, 32], DT.float32,
                                                    tag="t1b", name="m1p",
                                                    bufs=2)
                                    nc.vector.tensor_mul(m1p[:], gt[:, :, 0, :],
                                                         gt[:, :, 2, :])
                                    t1p = work.tile([128, 4, 32], DT.float32,
                                                    tag="tncb", name="t1p",
                                                    bufs=2)
                                    nc.vector.tensor_mul(
                                        t1p[:], gt[:, :, 1, :],
                                        cT_prev[:, 4 * h:4 * h + 4,
                                                r0:r0 + 32])
                                    ncs = cT_new[:, 4 * h:4 * h + 4,
                                                 r0:r0 + 32]
                                    nc.vector.tensor_add(ncs, t1p[:], m1p[:])
                                    tncp = work.tile([128, 4, 32], DT.float32,
                                                     tag="ptnc", name="tncp",
                                                     bufs=2)
                                    nc.scalar.activation(tncp[:], ncs, AF.Tanh)
                                    nhfp = work.tile([128, 4, 32], DT.float32,
                                                     tag="pnh", name="nhfp",
                                                     bufs=2)
                                    nc.vector.tensor_mul(nhfp[:],
                                                         gt[:, :, 3, :],
                                                         tncp[:])
                                    nc.vector.tensor_copy(
                                        nhT_new[:, 4 * h:4 * h + 4,
                                                r0:r0 + 32], nhfp[:])
                                    for c_ in range(4):
                                        nc.sync.dma_start(
                                            out=yt_ap[w, 4 * h + c_, :,
                                                      r0:r0 + 32],
                                            in_=nhfp[:, c_, :])
                            nhT_prev, cT_prev = nhT_new, cT_new

    nc.compile()
    return nc


# ---------------------------------------------------------------------------
# Entry point
# ---------------------------------------------------------------------------

_PROGRAM_CACHE = {}


def _run(inputs, trace=False):
    prep = _prep_inputs(**inputs)
    sch = prep["sch"]
    D, U, M, V, P = sch["D"], sch["U"], sch["M"], sch["V"], sch["P"]

    in_maps = []
    for c in range(NCORES):
        m = {
            "xt": prep["xt_blocks"][c],
            "wi": prep["Wi_l"],
            "wh": prep["Wh_l"],
            "bbc": prep["bbc"],
            "ident": prep["ident"],
            "identf": prep["identf"],
        }
        if prep["Wi8_l"] is not None:
            m["xt8"] = prep["xt8_blocks"][c]
            m["wi8"] = prep["Wi8_l"]
        if not prep["zero_init"]:
            m["ht0"] = prep["ht0_blocks"][c]
            m["c0"] = prep["c0_blocks"][c]
        in_maps.append(m)

    # Retry ladder: rare transient device errors have been observed on the
    # shared terminal; retry twice, then once more with the conservative
    # (no SBUF-resident tail waves) program variant.
    import time as _time
    res = None
    last_err = None
    no_tail_used = False
    for attempt, no_tail in enumerate([False, False, True]):
        key = (D, tuple(M.tolist()), tuple(U.tolist()), prep["Mzx"],
               prep["zx_row0"], prep["zero_init"], no_tail, PHASEA_F8)
        try:
            if key not in _PROGRAM_CACHE:
                _PROGRAM_CACHE[key] = _build_program(
                    D, U, M, V, P, prep["Mzx"], prep["zx_row0"],
                    prep["zx_start_tile"], prep["zero_init"], no_tail=no_tail)
            nc = _PROGRAM_CACHE[key]
            res = run_bass_kernel_spmd(nc, in_maps,
                                       core_ids=list(range(NCORES)),
                                       trace=trace)
            no_tail_used = no_tail
            break
        except Exception as e:  # noqa: BLE001 - retry on device hiccups
            last_err = e
            sys.stderr.write(f"kernel attempt {attempt} failed: {e!r}\n")
            trace = False  # profiling hook may be wedged; drop it on retry
            _time.sleep(2.0)
    if res is None:
        raise last_err

    T, B = prep["T"], prep["B"]
    dp = D if no_tail_used else _compute_dp(D, U, M, prep["zero_init"])
    rem_plan = [] if no_tail_used else _rem_plan(D, U, M, dp, prep["zero_init"])
    merge_rem = 0 if no_tail_used else _merge12(D, U, M, dp, prep["zero_init"])
    depth = sch["depth"]
    rank = sch["core_rank"][sch["chain_id"]]
    rem_mask = np.zeros(T * B, bool)
    for dd, rem in rem_plan:
        rb = (int(M[dd]) - 1) * 128
        rem_mask |= (depth == dd) & (rank >= rb)
    y_full = np.empty((T * B, H), np.float32)
    core_pos = prep["core_pos"]; padded_row = prep["padded_row"]
    if merge_rem > 0:
        # wave-1 remainder rows rode in wave 2's last-tile padding
        src0 = (int(M[1]) - 1) * 128
        mm = (depth == 1) & (rank >= src0)
        padded_row = padded_row.copy()
        ext0 = ((int(U[2]) + 15) // 16) * 16
        padded_row[mm] = int(P[2]) + ext0 + (rank[mm] - src0)
    for c in range(NCORES):
        selc = core_pos == c
        sel = selc & (depth < dp) & ~rem_mask
        y_full[sel] = res.results[c]["y"][padded_row[sel]].astype(np.float32)
        if rem_plan:
            yrr = res.results[c]["ytr"]  # [n_rem_slots, KT, 128, 32]
            yrr = yrr.reshape(yrr.shape[0], H, 32)
            slot = 0
            for dd, rem in rem_plan:
                rb = (int(M[dd]) - 1) * 128
                for p in range((rem + 31) // 32):
                    lo = rb + 32 * p
                    hi = min(lo + 32, rb + rem)
                    selr = selc & (depth == dd) & (rank >= lo) & (rank < hi)
                    if selr.any():
                        y_full[selr] = yrr[slot][:, rank[selr] - lo].T
                    slot += 1
        if dp < D:
            ytc = res.results[c]["yt"]  # [n_packed, KT, 128, 64]
            ytr = ytc.reshape(ytc.shape[0], H, 64)
            selt = selc & (depth >= dp)
            y_full[selt] = ytr[depth[selt] - dp, :, rank[selt]]
    return y_full, res


def kernel(**inputs) -> np.ndarray:
    y, _ = _run(inputs, trace=False)
    return y

